# revision 28
# baseline (speedup 1.0000x reference)
"""Trainium2 Bass kernel for single-head attention.

Problem: x[8, 2048, 512], Wq/Wk/Wv[512, 512], bq/bk/bv[512] ->
out[8, 2048, 512] where out = softmax((xWq+bq)(xWk+bk)^T / sqrt(512)) (xWv+bv).

Sharding: data-parallel over batch. Each of the 8 NeuronCores computes full
attention for one batch element.

Per-core algorithm (S=2048 sequence, D=512 hidden, P=128 partitions):
  1. Load x [S, D], transpose on the PE (128x128 blocks) to xT [D, S].
  2. QKV projections with the contraction dim (features) on partitions:
       qT/kT [D, S] = W.T @ xT   (bias fused into the PSUM->SBUF evacuation)
       v     [S, D] = x @ Wv     (natural layout; bias via broadcast add)
  3. Scores are computed TRANSPOSED, eT[j, i] = exp(scale * k_j . q_i), so
     that the softmax'd axis j lands on partitions - exactly what the
     attention*V matmul needs as its stationary operand. Softmax skips the
     max-subtraction (scaled scores are ~N(0,1); exp is safe in fp32).
  4. Denominators: eT tiles are accumulated over key-tiles on the DVE
     (esum), one ones-row matmul per query chunk gives den[1, i], the row is
     transposed into per-partition columns with K=1 matmuls, and the output
     tiles are scaled by 1/den during evacuation (split across DVE and ACT).
  5. out[i, h] accumulates sum_j eT[j, i] * v[j, h] over 16 j-tiles in PSUM.

All matmuls run in float32r (TF32-like fast fp32 mode, 4x the fp32 rate,
~1e-4 relative error), accumulating in fp32 PSUM.
"""

import os
import sys

for _p in ("/opt/trn_rl_repo", "/root/.axon_site/_ro/trn_rl_repo"):
    if os.path.isdir(_p) and _p not in sys.path:
        sys.path.append(_p)

import numpy as np

import concourse.bacc as bacc
import concourse.mybir as mybir
import concourse.tile as tile
from concourse.bass_utils import run_bass_kernel_spmd
from concourse.masks import make_identity

B = 8
S = 2048
D = 512
P = 128
NT = S // P  # 16 s-tiles (query/key tiles of 128)
FC = D // P  # 4 feature/hidden chunks of 128
MC = 4  # i-chunks of 512 queries
SCALE = 1.0 / float(np.sqrt(D))

F32 = mybir.dt.float32
F32R = mybir.dt.float32r
ACT_ID = mybir.ActivationFunctionType.Identity
ACT_EXP = mybir.ActivationFunctionType.Exp

_CACHE = {}


def _emit_body(nc, tc, pools, dram, rep, phases="all"):
    """Emit one full attention computation. `rep` uniquifies tile names."""
    consts, w_r_pool, qkT_pool, v_pool, e_pool, out_pool, den_pool = pools
    x, wq, bq, wk, bk, wv, bv, out = dram

    ident = consts["ident"]
    ones32 = consts["ones32"]
    ones_r = consts["ones_r"]
    bq_sb = consts["bq_sb"]
    bk_sb = consts["bk_sb"]
    bv_sb = consts["bv_sb"]
    wq_r = consts["wq_r"]

    # qT_r[c][j] / kT_r[c][j]: hidden-chunk c (128 partitions), query-chunk j
    # (512). Fine-grained tiles so scores can start before all of QKV is done.
    qT_r = [
        [
            qkT_pool.tile([P, 512], F32R, tag=f"qT{c}_{j}", name=f"qT{c}_{j}_{rep}")
            for j in range(FC)
        ]
        for c in range(FC)
    ]
    kT_r = [
        [
            qkT_pool.tile([P, 512], F32R, tag=f"kT{c}_{j}", name=f"kT{c}_{j}_{rep}")
            for j in range(FC)
        ]
        for c in range(FC)
    ]
    v_r = [
        v_pool.tile([P, D], F32R, tag=f"v{t}", name=f"v{t}_{rep}") for t in range(NT)
    ]

    with (
        tc.tile_pool(name="xstage", bufs=2) as xstage,
        tc.tile_pool(name="xT", bufs=2) as xT_pool,
        tc.tile_pool(name="psT", bufs=4, space="PSUM") as psT,
        tc.tile_pool(name="psQKV", bufs=3, space="PSUM") as psQKV,
    ):
        # per s-chunk jj (4 s-tiles = 512 rows): DMA in, transpose, then
        # immediately emit the QKV matmuls that consume just this chunk.
        for jj in range(FC):
            stage = xstage.tile([P, 4, D], F32, tag="stage", name=f"stage{jj}_{rep}")
            nc.sync.dma_start(
                out=stage[:],
                in_=x[jj * 4 * P : (jj + 1) * 4 * P, :].rearrange(
                    "(t p) f -> p t f", p=P
                ),
            )
            if "wk_r" not in consts:
                consts["load_rest"]()
            wk_r = consts["wk_r"]
            wv_r = consts["wv_r"]
            xT_c = [
                xT_pool.tile([P, 512], F32R, tag=f"xT{c}", name=f"xT{c}_{jj}_{rep}")
                for c in range(FC)
            ]
            for tl in range(4):
                for c in range(FC):
                    pst = psT.tile([P, P], F32, tag="pst", name=f"pst{jj}_{tl}_{c}_{rep}")
                    nc.tensor.transpose(
                        pst[:], stage[:, tl, c * P : (c + 1) * P], ident[:]
                    )
                    nc.any.tensor_copy(xT_c[c][:, tl * P : (tl + 1) * P], pst[:])

            # Q^T, K^T for this s-chunk: [h-tile 128, 512]
            for wr, dst, bias in ((wq_r, qT_r, bq_sb), (wk_r, kT_r, bk_sb)):
                for i in range(FC):  # h-tile
                    ps = psQKV.tile(
                        [P, 512], F32, tag="psqk", name=f"psqk{i}_{jj}_{rep}"
                    )
                    for c in range(FC):  # contraction over features
                        nc.tensor.matmul(
                            ps[:],
                            wr[:, c, i * P : (i + 1) * P],
                            xT_c[c][:],
                            start=(c == 0),
                            stop=(c == FC - 1),
                        )
                    nc.scalar.activation(
                        dst[i][jj][:],
                        ps[:],
                        ACT_ID,
                        bias=bias[:, i : i + 1],
                    )

            # V for these 4 s-tiles: [s-tile 128, h]
            for tl in range(4):
                t = jj * 4 + tl
                ps = psQKV.tile([P, 512], F32, tag="psqk", name=f"psv{t}_{rep}")
                for c in range(FC):
                    nc.tensor.matmul(
                        ps[:],
                        xT_c[c][:, tl * P : (tl + 1) * P],
                        wv_r[:, c, :],
                        start=(c == 0),
                        stop=(c == FC - 1),
                    )
                nc.vector.tensor_add(v_r[t][:], ps[:], bv_sb[:])

    if phases == "qkv":
        # ablation: write q/k/v straight out
        for t in range(4):
            o_sb = out_pool.tile([P, D], F32, tag="osb", name=f"oq{t}_{rep}")
            nc.vector.tensor_copy(o_sb[:], v_r[t][:])
            nc.sync.dma_start(out=out[t * P : (t + 1) * P, :], in_=o_sb[:])
        return

    # ---- scores^T -> exp -> denominators + attention * V ----
    with (
        tc.tile_pool(name="psS", bufs=3, space="PSUM") as psS,
        tc.tile_pool(name="psO", bufs=1, space="PSUM") as psO,
        tc.tile_pool(name="psDen", bufs=1, space="PSUM") as psDen,
    ):
        psDenT = psDen
        for m in range(MC):  # chunk of 512 queries
            ps_o = [
                psO.tile([P, D], F32, tag=f"o{t}", name=f"ps_o{t}_{m}_{rep}")
                for t in range(4)
            ]
            ps_den = psDen.tile([1, 512], F32, tag="ps_den", name=f"ps_den{m}_{rep}")
            esum = den_pool.tile([P, 512], F32R, tag="esum", name=f"esum{m}_{rep}")
            for c in range(NT):  # key tile of 128
                ps_s = psS.tile([P, 512], F32, tag="ps_s", name=f"ps_s{m}_{c}_{rep}")
                for hc in range(FC):  # contraction over hidden
                    nc.tensor.matmul(
                        ps_s[:],
                        kT_r[hc][c // 4][:, (c % 4) * P : (c % 4 + 1) * P],
                        qT_r[hc][m][:],
                        start=(hc == 0),
                        stop=(hc == FC - 1),
                    )
                eT = e_pool.tile([P, 512], F32R, tag="eT", name=f"eT{m}_{c}_{rep}")
                nc.scalar.activation(eT[:], ps_s[:], ACT_EXP, scale=SCALE)
                # accumulate eT over key tiles on the (otherwise idle) DVE;
                # one ones-matmul per m-chunk then yields the denominators.
                if c == 0:
                    nc.vector.tensor_copy(esum[:], eT[:])
                else:
                    nc.vector.tensor_add(esum[:], esum[:], eT[:])
                # out[i, h] += eT[j, i-tile].T @ v[j, h]
                for t in range(4):
                    nc.tensor.matmul(
                        ps_o[t][:],
                        eT[:, t * P : (t + 1) * P],
                        v_r[c][:],
                        start=(c == 0),
                        stop=(c == NT - 1),
                    )

            # denominator row: den[1, i] = sum_j esum[j, i]
            nc.tensor.matmul(ps_den[:], ones_r[:], esum[:], start=True, stop=True)
            # transpose the denominator row into per-partition columns with
            # K=1 matmuls, then reciprocal.
            den_row = den_pool.tile([1, 512], F32, tag="den_row", name=f"dr{m}_{rep}")
            nc.vector.tensor_copy(den_row[:], ps_den[:])
            ps_denT = psDenT.tile([P, 4], F32, tag="ps_den", name=f"ps_denT{m}_{rep}")
            for t in range(4):
                nc.tensor.matmul(
                    ps_denT[:, t : t + 1],
                    den_row[:, t * P : (t + 1) * P],
                    ones32[:1, :],
                    start=True,
                    stop=True,
                )
            rec = den_pool.tile([P, 4], F32, tag="rec", name=f"rec{m}_{rep}")
            nc.vector.reciprocal(rec[:], ps_denT[:])

            for t in range(4):
                o_sb = out_pool.tile([P, D], F32, tag="osb", name=f"o{m}_{t}_{rep}")
                if t < 2:
                    nc.vector.tensor_scalar_mul(
                        o_sb[:], ps_o[t][:], rec[:, t : t + 1]
                    )
                else:
                    nc.scalar.activation(
                        o_sb[:], ps_o[t][:], ACT_ID, scale=rec[:, t : t + 1]
                    )
                it = m * 4 + t
                nc.sync.dma_start(out=out[it * P : (it + 1) * P, :], in_=o_sb[:])


def _build_nc(reps=1, phases="all"):
    nc = bacc.Bacc(None)

    x = nc.dram_tensor("x", [S, D], F32, kind="ExternalInput")
    wq = nc.dram_tensor("Wq", [D, D], F32, kind="ExternalInput")
    bq = nc.dram_tensor("bq", [D], F32, kind="ExternalInput")
    wk = nc.dram_tensor("Wk", [D, D], F32, kind="ExternalInput")
    bk = nc.dram_tensor("bk", [D], F32, kind="ExternalInput")
    wv = nc.dram_tensor("Wv", [D, D], F32, kind="ExternalInput")
    bv = nc.dram_tensor("bv", [D], F32, kind="ExternalInput")
    out = nc.dram_tensor("out", [S, D], F32, kind="ExternalOutput")
    dram = (x, wq, bq, wk, bk, wv, bv, out)

    with tile.TileContext(nc) as tc:
        with (
            tc.tile_pool(name="consts", bufs=1) as consts_pool,
            tc.tile_pool(name="w_r", bufs=1) as w_r_pool,
            tc.tile_pool(name="qkT", bufs=1) as qkT_pool,
            tc.tile_pool(name="v", bufs=1) as v_pool,
            tc.tile_pool(name="e", bufs=4) as e_pool,
            tc.tile_pool(name="outsb", bufs=3) as out_pool,
            tc.tile_pool(name="den", bufs=2) as den_pool,
        ):
            consts = {}
            ident = consts_pool.tile([P, P], F32, tag="ident", name="ident")
            make_identity(nc, ident[:])
            consts["ident"] = ident

            ones32 = consts_pool.tile([P, 1], F32, tag="ones32", name="ones32")
            nc.vector.memset(ones32[:], 1.0)
            ones_r = consts_pool.tile([P, 1], F32R, tag="ones_r", name="ones_r")
            nc.vector.tensor_copy(ones_r[:], ones32[:])
            consts["ones32"] = ones32
            consts["ones_r"] = ones_r

            bq_sb = consts_pool.tile([P, FC], F32, tag="bq", name="bq_sb")
            bk_sb = consts_pool.tile([P, FC], F32, tag="bk", name="bk_sb")
            nc.gpsimd.dma_start(out=bq_sb[:], in_=bq.rearrange("(c p) -> p c", p=P))
            nc.gpsimd.dma_start(out=bk_sb[:], in_=bk.rearrange("(c p) -> p c", p=P))
            bv_sb = consts_pool.tile([P, D], F32, tag="bv", name="bv_sb")
            nc.gpsimd.dma_start(out=bv_sb[:], in_=bv[:].partition_broadcast(P))
            consts["bq_sb"] = bq_sb
            consts["bk_sb"] = bk_sb
            consts["bv_sb"] = bv_sb

            wstage_cm = tc.tile_pool(name="wstage", bufs=1)
            wstage = wstage_cm.__enter__()

            def _load_w(wi, wname, wdram):
                stage = wstage.tile(
                    [P, FC, D], F32, tag="wstage", name=f"wstage{wi}"
                )
                nc.sync.dma_start(
                    out=stage[:], in_=wdram.rearrange("(c p) h -> p c h", p=P)
                )
                wr = w_r_pool.tile([P, FC, D], F32R, tag=f"w{wi}", name=f"w{wi}")
                nc.vector.tensor_copy(wr[:], stage[:])
                consts[wname] = wr

            _load_w(0, "wq_r", wq)

            def _load_rest():
                _load_w(1, "wk_r", wk)
                _load_w(2, "wv_r", wv)

            consts["load_rest"] = _load_rest

            pools = (
                consts, w_r_pool, qkT_pool, v_pool, e_pool, out_pool, den_pool,
            )
            for rep in range(reps):
                _emit_body(nc, tc, pools, dram, rep, phases=phases)
            wstage_cm.__exit__(None, None, None)

    nc.finalize()
    return nc


def kernel(x, Wq, bq, Wk, bk, Wv, bv):
    x = np.ascontiguousarray(np.asarray(x, dtype=np.float32))
    args = {
        "Wq": np.ascontiguousarray(np.asarray(Wq, dtype=np.float32)),
        "bq": np.ascontiguousarray(np.asarray(bq, dtype=np.float32)),
        "Wk": np.ascontiguousarray(np.asarray(Wk, dtype=np.float32)),
        "bk": np.ascontiguousarray(np.asarray(bk, dtype=np.float32)),
        "Wv": np.ascontiguousarray(np.asarray(Wv, dtype=np.float32)),
        "bv": np.ascontiguousarray(np.asarray(bv, dtype=np.float32)),
    }

    if "nc" not in _CACHE:
        _CACHE["nc"] = _build_nc()
    nc = _CACHE["nc"]

    in_maps = [{"x": x[b], **args} for b in range(B)]
    try:
        res = run_bass_kernel_spmd(nc, in_maps, list(range(B)))
    except Exception:
        # transient device wedge (e.g. NRT_EXEC_UNIT_UNRECOVERABLE) - retry
        import time as _time

        _time.sleep(5)
        res = run_bass_kernel_spmd(nc, in_maps, list(range(B)))
    return np.stack([res.results[b]["out"] for b in range(B)]).astype(np.float32)


if __name__ == "__main__":
    rng = np.random.default_rng(0)
    inputs = {
        "x": rng.standard_normal((B, S, D), dtype=np.float32),
        "Wq": rng.standard_normal((D, D), dtype=np.float32) / np.sqrt(D),
        "bq": rng.standard_normal(D).astype(np.float32) * 0.01,
        "Wk": rng.standard_normal((D, D), dtype=np.float32) / np.sqrt(D),
        "bk": rng.standard_normal(D).astype(np.float32) * 0.01,
        "Wv": rng.standard_normal((D, D), dtype=np.float32) / np.sqrt(D),
        "bv": rng.standard_normal(D).astype(np.float32) * 0.01,
    }
    got = kernel(**inputs)
    print("kernel output", got.shape, got.dtype)
